# revision 1
# baseline (speedup 1.0000x reference)
"""Multi-region RNN kernel for Trainium2 (8 NeuronCores, SPMD batch-sharded).

Model (per step t):
    inp  = einsum('bi,rih->rbh', x_t, W_ih)
    loc  = einsum('rbh,rhg->rbg', H, W_hh)
    msg  = einsum('ij,ibh->jbh', C, H)
    cross= einsum('rbh,rhg->rbg', msg, W_rhh)
    H'   = tanh(inp + loc + cross + bias)
Output: stack H over t -> [T,B,R*H] @ W_out + b_out.

Distribution: pure data-parallel over batch (B=32 -> 4 per core), parameters
replicated; no cross-core communication. Per core:
  Phase 1: input drive for all t precomputed as per-region matmuls
           (W_ih[r] stationary, x^T moving), bias folded in, staged to DRAM
           in [t][h,(b,r)] bf16 layout.
  Phase 2: sequential recurrence, latency-chain optimized. State as bf16
           [h=128,(b,r)=512] tiles. The region-major copy needed by the msg
           matmuls is produced ON-CHIP: PE transpose-mode ops on the bf16
           state (per b) into PSUM, evacuated by the DVE -- no DMA transposes
           on the critical path. The input drive is accumulated directly into
           the loc/cross PSUM bank by a single identity matmul (pa += I@inp),
           so there is no DVE add pass, and tanh reads PSUM directly.
           Per-step chain: tanh(ACT, r-halves) -> transpose(PE) -> Hrm copy
           (DVE) -> msg(PE) -> Msg copy(DVE, r-halves) -> cross(PE) ->
           ident-matmul -> tanh. loc matmuls fill the gaps; the input loads
           ride the sync HWDGE queue and history stores the scalar HWDGE
           queue, keeping the gpsimd SWDGE queue out of the chain.
  Phase 3: output projection from the bf16 state history (DRAM) with
           per-region accumulation into PSUM over (t,b)-blocks of 128;
           b_out added via a K=1 matmul of ones x b_out.
"""

import numpy as np
import ml_dtypes
from contextlib import ExitStack

import concourse.bass as bass
import concourse.bacc as bacc
import concourse.tile as tile
from concourse import mybir
from concourse.bass_utils import run_bass_kernel_spmd
from concourse.masks import make_identity

T, B, I, H, R, O = 128, 32, 128, 128, 100, 64
NCORES = 8
BL = B // NCORES          # batch per core = 4
BR = BL * R               # state free size = 400, col = b*R + r
TB = T * BL               # 512
TBLK = 32                 # t-steps per phase-1/3 block -> 128 (t,b) cols
RPAD = 128                # region stride in state layout
BRP = BL * RPAD           # padded state free size = 512, col = b*RPAD + r

BF = mybir.dt.bfloat16
F32 = mybir.dt.float32
Act = mybir.ActivationFunctionType

_CACHE: dict = {}
NREP = 1   # test-only hook: repeat the whole body to measure device time deltas


def _build_program():
    nc = bacc.Bacc(None, target_bir_lowering=False)

    xT_d = nc.dram_tensor("xT", [I, TB], BF, kind="ExternalInput")        # [i,(t,b)]
    C_d = nc.dram_tensor("C", [R, R], BF, kind="ExternalInput")           # [i,j]
    Whh_d = nc.dram_tensor("Whh", [H, R * H], BF, kind="ExternalInput")   # [h,(r,g)]
    Wrhh_d = nc.dram_tensor("Wrhh", [H, R * H], BF, kind="ExternalInput")
    Wih_d = nc.dram_tensor("Wih", [I, R * H], BF, kind="ExternalInput")
    Wout_d = nc.dram_tensor("Wout", [H, R * O], BF, kind="ExternalInput")  # [h,(r,o)]
    biasT_d = nc.dram_tensor("biasT", [H, R], F32, kind="ExternalInput")
    bout_d = nc.dram_tensor("bout", [1, O], BF, kind="ExternalInput")
    out_d = nc.dram_tensor("out", [T, BL, O], F32, kind="ExternalOutput")

    with tile.TileContext(nc) as tc, ExitStack() as ctx:
        consts = ctx.enter_context(tc.tile_pool(name="consts", bufs=1))
        dram = ctx.enter_context(tc.tile_pool(name="dram", bufs=1, space="DRAM"))

        Whh_s = consts.tile([H, R * H], BF)
        nc.sync.dma_start(Whh_s[:], Whh_d[:])
        Wrhh_s = consts.tile([H, R * H], BF)
        nc.sync.dma_start(Wrhh_s[:], Wrhh_d[:])
        Wih_s = consts.tile([I, R * H], BF)
        nc.sync.dma_start(Wih_s[:], Wih_d[:])
        Wout_s = consts.tile([H, R * O], BF)
        nc.sync.dma_start(Wout_s[:], Wout_d[:])
        xT_s = consts.tile([I, TB], BF)
        nc.sync.dma_start(xT_s[:], xT_d[:])
        C_s = consts.tile([R, R], BF)
        nc.sync.dma_start(C_s[:], C_d[:])
        biasT_s = consts.tile([H, R], F32)
        nc.sync.dma_start(biasT_s[:], biasT_d[:])
        bout_s = consts.tile([1, O], BF)
        nc.sync.dma_start(bout_s[:], bout_d[:])
        ones_s = consts.tile([1, H], BF)
        nc.vector.memset(ones_s[:], 1.0)
        identB_s = consts.tile([H, H], BF)
        make_identity(nc, identB_s[:])

        def _emit_body(_rep, bctx):
            inp_dram = dram.tile([T, H, BRP], BF, name=f"inp_dram{_rep}")
            hist_dram = dram.tile([T, H, BRP], BF, name=f"hist_dram{_rep}")
            inp4d = inp_dram.rearrange("t h (b r) -> t h b r", r=RPAD)
            hist4d = hist_dram.rearrange("t h (b r) -> t h b r", r=RPAD)

            # ---------------- Phase 1: input drive ----------------
            NBLK = T // TBLK
            with ExitStack() as p1ctx:
                p1_ps = p1ctx.enter_context(
                    tc.tile_pool(name=f"p1_ps{_rep}", bufs=2, space="PSUM"))
                p1_st = p1ctx.enter_context(tc.tile_pool(name=f"p1_st{_rep}", bufs=1))
                for tb in range(NBLK):
                    stage = p1_st.tile([H, TBLK * BR], BF, tag="p1stage")
                    stage4 = stage.rearrange("h (t b r) -> h t b r", b=BL, r=R)
                    for r in range(R):
                        ps = p1_ps.tile([H, TBLK * BL], F32, tag="p1psum")
                        nc.tensor.matmul(
                            ps[:],
                            Wih_s[:, r * H:(r + 1) * H],
                            xT_s[:, tb * TBLK * BL:(tb + 1) * TBLK * BL],
                            start=True, stop=True,
                        )
                        nc.scalar.activation(
                            out=stage4[:, :, :, r],
                            in_=ps.rearrange("h (t b) -> h t b", b=BL),
                            func=Act.Identity,
                            bias=biasT_s[:, r:r + 1],
                            scale=1.0,
                        )
                    for b in range(BL):
                        nc.sync.dma_start(
                            out=inp4d[tb * TBLK:(tb + 1) * TBLK, :, b, 0:R].rearrange(
                                "t h r -> h t r"),
                            in_=stage4[:, :, b, :],
                        )

            # ---------------- Phase 2: recurrence ----------------
            with ExitStack() as p2ctx:
                st_pool = p2ctx.enter_context(tc.tile_pool(name=f"st{_rep}", bufs=3))
                hrm_pool = p2ctx.enter_context(tc.tile_pool(name=f"hrm{_rep}", bufs=2))
                msg_pool = p2ctx.enter_context(tc.tile_pool(name=f"msgp{_rep}", bufs=2))
                inp_pool = p2ctx.enter_context(tc.tile_pool(name=f"inpp{_rep}", bufs=3))
                ps_act = p2ctx.enter_context(
                    tc.tile_pool(name=f"ps_act{_rep}", bufs=2, space="PSUM"))
                ps_msg = p2ctx.enter_context(
                    tc.tile_pool(name=f"ps_msg{_rep}", bufs=2, space="PSUM"))
                ps_pret = p2ctx.enter_context(
                    tc.tile_pool(name=f"ps_pret{_rep}", bufs=2, space="PSUM"))

                Hprev = st_pool.tile([H, BRP], BF, tag="hstate")
                nc.vector.memset(Hprev[:], 0.0)

                for t in range(T):
                    inp_t = inp_pool.tile([H, BRP], BF, tag="inp_t")
                    nc.sync.dma_start(inp_t[:], inp_dram[t, :, :])

                    pa = ps_act.tile([H, BRP], F32, tag="pa")
                    paR = pa.rearrange("h (b r) -> h b r", r=RPAD)
                    HprevR = Hprev.rearrange("h (b r) -> h b r", r=RPAD)

                    # local recurrence, first chunk (only needs the first
                    # tanh half of the previous step).
                    # PSUM start=True clears the whole bank's has_written flags,
                    # so only the first matmul into pa may set it.
                    for r in range(0, 64):
                        nc.tensor.matmul(
                            paR[:, :, r],
                            Whh_s[:, r * H:(r + 1) * H],
                            HprevR[:, :, r],
                            start=(r == 0), stop=False,
                        )

                    # region-major state for msg: PE transpose-mode of the
                    # previous bf16 state, evacuated to SBUF by the DVE
                    Hrm = hrm_pool.tile([RPAD, BL * H], BF, tag="hrm")
                    if t == 0:
                        nc.vector.memset(Hrm[:], 0.0)
                    else:
                        pret = ps_pret.tile([RPAD, BL * H], BF, tag="pret")
                        for b in range(BL):
                            nc.tensor.transpose(
                                pret[:, b * H:(b + 1) * H],
                                Hprev[:, b * RPAD:(b + 1) * RPAD],
                                identB_s[:],
                            )
                        for half in range(2):
                            nc.vector.tensor_copy(
                                out=Hrm[:, half * 2 * H:(half + 1) * 2 * H],
                                in_=pret[:, half * 2 * H:(half + 1) * 2 * H])

                    # message matmuls: pm[h,(b,j)] = Hrm_b^T @ C per b
                    pm = ps_msg.tile([H, BRP], F32, tag="pm")
                    for b in range(BL):
                        nc.tensor.matmul(
                            pm[:, b * RPAD:b * RPAD + R],
                            Hrm[0:R, b * H:(b + 1) * H],
                            C_s[:],
                            start=(b == 0), stop=(b == BL - 1),
                        )

                    for r in range(64, R):
                        nc.tensor.matmul(
                            paR[:, :, r],
                            Whh_s[:, r * H:(r + 1) * H],
                            HprevR[:, :, r],
                            start=False, stop=False,
                        )

                    # PSUM->SBUF copy of msg on DVE in r-halves so the cross
                    # matmuls can start as soon as the first half lands
                    Msg = msg_pool.tile([H, BRP], BF, tag="msg")
                    MsgR = Msg.rearrange("h (b r) -> h b r", r=RPAD)
                    pmR = pm.rearrange("h (b r) -> h b r", r=RPAD)
                    for lo, hi in ((0, 64), (64, RPAD)):
                        nc.vector.tensor_copy(out=MsgR[:, :, lo:hi],
                                              in_=pmR[:, :, lo:hi])
                        for r in range(lo, min(hi, R)):
                            nc.tensor.matmul(
                                paR[:, :, r],
                                Wrhh_s[:, r * H:(r + 1) * H],
                                MsgR[:, :, r],
                                start=False, stop=False,
                            )

                    # input drive: accumulated straight into PSUM by the PE
                    # (pa += I @ inp), so no separate DVE add pass is needed
                    nc.tensor.matmul(pa[:], identB_s[:], inp_t[:],
                                     start=False, stop=True)

                    # state tanh in r-halves straight from PSUM: unblocks the
                    # next step's loc chunk early
                    Hnext = st_pool.tile([H, BRP], BF, tag="hstate")
                    HnextR = Hnext.rearrange("h (b r) -> h b r", r=RPAD)
                    for lo, hi in ((0, 64), (64, RPAD)):
                        nc.scalar.activation(out=HnextR[:, :, lo:hi],
                                             in_=paR[:, :, lo:hi], func=Act.Tanh)
                    nc.scalar.dma_start(out=hist_dram[t, :, :], in_=Hnext[:])
                    Hprev = Hnext

            # ---------------- Phase 3: output projection ----------------
            with ExitStack() as p3ctx:
                p3_hh = p3ctx.enter_context(tc.tile_pool(name=f"p3_hh{_rep}", bufs=2))
                p3_ps = p3ctx.enter_context(
                    tc.tile_pool(name=f"p3_ps{_rep}", bufs=2, space="PSUM"))
                p3_ot = p3ctx.enter_context(tc.tile_pool(name=f"p3_ot{_rep}", bufs=2))
                for g in range(NBLK):
                    hh = p3_hh.tile([H, TBLK * BR], BF, tag="hh")
                    hh4 = hh.rearrange("h (t b r) -> h t b r", b=BL, r=R)
                    for b in range(BL):
                        nc.sync.dma_start(
                            out=hh4[:, :, b, :],
                            in_=hist4d[g * TBLK:(g + 1) * TBLK, :, b, 0:R].rearrange(
                                "t h r -> h t r"),
                        )
                    po = p3_ps.tile([TBLK * BL, O], F32, tag="po")
                    for r in range(R):
                        nc.tensor.matmul(
                            po[:],
                            hh4[:, :, :, r],
                            Wout_s[:, r * O:(r + 1) * O],
                            start=(r == 0), stop=False,
                        )
                    nc.tensor.matmul(po[:], ones_s[:, 0:TBLK * BL], bout_s[:],
                                     start=False, stop=True)
                    ot = p3_ot.tile([TBLK * BL, O], F32, tag="ot")
                    nc.scalar.activation(out=ot[:], in_=po[:], func=Act.Copy, scale=1.0)
                    nc.sync.dma_start(
                        out=out_d[g * TBLK:(g + 1) * TBLK, :, :].rearrange(
                            "t b o -> (t b) o"),
                        in_=ot[:],
                    )

        for _rep in range(NREP):
            with ExitStack() as bctx:
                _emit_body(_rep, bctx)

    nc.compile()
    return nc


def _prep_inputs(x, C, W_ih, W_hh, W_rhh, bias, W_out, b_out):
    bf = ml_dtypes.bfloat16
    shared = {
        "C": np.ascontiguousarray(C).astype(bf),
        "Whh": np.ascontiguousarray(W_hh.transpose(1, 0, 2).reshape(H, R * H)).astype(bf),
        "Wrhh": np.ascontiguousarray(W_rhh.transpose(1, 0, 2).reshape(H, R * H)).astype(bf),
        "Wih": np.ascontiguousarray(W_ih.transpose(1, 0, 2).reshape(I, R * H)).astype(bf),
        "Wout": np.ascontiguousarray(
            W_out.reshape(R, H, O).transpose(1, 0, 2).reshape(H, R * O)
        ).astype(bf),
        "biasT": np.ascontiguousarray(bias.T).astype(np.float32),
        "bout": np.ascontiguousarray(b_out.reshape(1, O)).astype(bf),
    }
    in_maps = []
    for c in range(NCORES):
        xc = x[:, c * BL:(c + 1) * BL, :]                     # [T, BL, I]
        xT = np.ascontiguousarray(xc.transpose(2, 0, 1).reshape(I, TB)).astype(bf)
        m = dict(shared)
        m["xT"] = xT
        in_maps.append(m)
    return in_maps


def kernel(x, C, W_ih, W_hh, W_rhh, bias, W_out, b_out, _trace=False):
    x = np.asarray(x, np.float32)
    in_maps = _prep_inputs(
        x, np.asarray(C, np.float32), np.asarray(W_ih, np.float32),
        np.asarray(W_hh, np.float32), np.asarray(W_rhh, np.float32),
        np.asarray(bias, np.float32), np.asarray(W_out, np.float32),
        np.asarray(b_out, np.float32),
    )
    if "nc" not in _CACHE:
        _CACHE["nc"] = _build_program()
    nc = _CACHE["nc"]
    res = run_bass_kernel_spmd(nc, in_maps, list(range(NCORES)), trace=_trace)
    out = np.empty((T, B, O), np.float32)
    for c in range(NCORES):
        out[:, c * BL:(c + 1) * BL, :] = res.results[c]["out"]
    if _trace:
        return out, res
    return out



# revision 5
# speedup vs baseline: 1806.6748x; 1806.6748x over previous
"""Multi-region RNN kernel for Trainium2 (8 NeuronCores, SPMD batch-sharded).

Model (per step t):
    inp  = einsum('bi,rih->rbh', x_t, W_ih)
    loc  = einsum('rbh,rhg->rbg', H, W_hh)
    msg  = einsum('ij,ibh->jbh', C, H)
    cross= einsum('rbh,rhg->rbg', msg, W_rhh)
    H'   = tanh(inp + loc + cross + bias)
Output: stack H over t -> [T,B,R*H] @ W_out + b_out.

Distribution: pure data-parallel over batch (B=32 -> 4 per core), parameters
replicated; no cross-core communication. Per core:
  Phase 1: input drive for all t precomputed as per-region matmuls
           (W_ih[r] stationary, x^T moving), bias folded in, staged to DRAM
           in [t][h,(b,r)] bf16 layout.
  Phase 2: sequential recurrence, latency-chain optimized. State as bf16
           [h=128,(b,r)=512] tiles. The region-major copy needed by the msg
           matmuls is produced ON-CHIP: PE transpose-mode ops on the bf16
           state (per b) into PSUM, evacuated by the DVE -- no DMA transposes
           on the critical path. The input drive is accumulated directly into
           the loc/cross PSUM bank by a single identity matmul (pa += I@inp),
           so there is no DVE add pass, and tanh reads PSUM directly.
           Per-step chain: tanh(ACT, r-halves) -> transpose(PE) -> Hrm copy
           (DVE) -> msg(PE) -> Msg copy(DVE, r-halves) -> cross(PE) ->
           ident-matmul -> tanh. loc matmuls fill the gaps; the input loads
           ride the sync HWDGE queue and history stores the scalar HWDGE
           queue, keeping the gpsimd SWDGE queue out of the chain.
  Phase 3: output projection from the bf16 state history (DRAM) with
           per-region accumulation into PSUM over (t,b)-blocks of 128;
           b_out added via a K=1 matmul of ones x b_out.
"""

import numpy as np
import ml_dtypes
from contextlib import ExitStack

import concourse.bass as bass
import concourse.bacc as bacc
import concourse.tile as tile
from concourse import mybir
from concourse.bass_utils import run_bass_kernel_spmd
from concourse.masks import make_identity

T, B, I, H, R, O = 128, 32, 128, 128, 100, 64
NCORES = 8
BL = B // NCORES          # batch per core = 4
BR = BL * R               # state free size = 400, col = b*R + r
TB = T * BL               # 512
TBLK = 32                 # t-steps per phase-1/3 block -> 128 (t,b) cols
RPAD = 128                # region stride in state layout
BRP = BL * RPAD           # padded state free size = 512, col = b*RPAD + r

BF = mybir.dt.bfloat16
F8 = mybir.dt.float8e4
F32 = mybir.dt.float32
WSCALE = 64.0
Act = mybir.ActivationFunctionType

_CACHE: dict = {}
NREP = 1   # test-only hook: repeat the whole body to measure device time deltas


def _build_program():
    nc = bacc.Bacc(None, target_bir_lowering=False)

    xT_d = nc.dram_tensor("xT", [I, TB], BF, kind="ExternalInput")        # [i,(t,b)]
    C_d = nc.dram_tensor("C", [R, R], BF, kind="ExternalInput")           # [i,j]
    Whh_d = nc.dram_tensor("Whh", [H, R * H], BF, kind="ExternalInput")   # [h,(r,g)]
    Wrhh_d = nc.dram_tensor("Wrhh", [H, R * H], F8, kind="ExternalInput")
    Wih_d = nc.dram_tensor("Wih", [I, R * H], BF, kind="ExternalInput")
    Wout_d = nc.dram_tensor("Wout", [H, R * O], BF, kind="ExternalInput")  # [h,(r,o)]
    biasT_d = nc.dram_tensor("biasT", [H, R], F32, kind="ExternalInput")
    bout_d = nc.dram_tensor("bout", [1, O], BF, kind="ExternalInput")
    out_d = nc.dram_tensor("out", [T, BL, O], F32, kind="ExternalOutput")

    with tile.TileContext(nc) as tc, ExitStack() as ctx:
        consts = ctx.enter_context(tc.tile_pool(name="consts", bufs=1))
        dram = ctx.enter_context(tc.tile_pool(name="dram", bufs=1, space="DRAM"))

        Whh_s = consts.tile([H, R * H], BF)
        nc.sync.dma_start(Whh_s[:], Whh_d[:])
        Wrhh_s = consts.tile([H, R * H], F8)
        nc.sync.dma_start(Wrhh_s[:], Wrhh_d[:])
        Wih_s = consts.tile([I, R * H], BF)
        nc.sync.dma_start(Wih_s[:], Wih_d[:])
        Wout_s = consts.tile([H, R * O], BF)
        nc.sync.dma_start(Wout_s[:], Wout_d[:])
        xT_s = consts.tile([I, TB], BF)
        nc.sync.dma_start(xT_s[:], xT_d[:])
        C_s = consts.tile([R, R], BF)
        nc.sync.dma_start(C_s[:], C_d[:])
        biasT_s = consts.tile([H, R], F32)
        nc.sync.dma_start(biasT_s[:], biasT_d[:])
        bout_s = consts.tile([1, O], BF)
        nc.sync.dma_start(bout_s[:], bout_d[:])
        ones_s = consts.tile([1, H], BF)
        nc.vector.memset(ones_s[:], 1.0)
        identB_s = consts.tile([H, H], BF)
        make_identity(nc, identB_s[:])

        def _emit_body(_rep, bctx):
            inp_dram = dram.tile([T, H, BRP], BF, name=f"inp_dram{_rep}")
            hist_dram = dram.tile([T, H, BRP], BF, name=f"hist_dram{_rep}")
            inp4d = inp_dram.rearrange("t h (b r) -> t h b r", r=RPAD)
            hist4d = hist_dram.rearrange("t h (b r) -> t h b r", r=RPAD)

            # ---------------- Phase 1: input drive ----------------
            NBLK = T // TBLK
            with ExitStack() as p1ctx:
                p1_ps = p1ctx.enter_context(
                    tc.tile_pool(name=f"p1_ps{_rep}", bufs=2, space="PSUM"))
                p1_st = p1ctx.enter_context(tc.tile_pool(name=f"p1_st{_rep}", bufs=1))
                for tb in range(NBLK):
                    stage = p1_st.tile([H, TBLK * BR], BF, tag="p1stage")
                    stage4 = stage.rearrange("h (t b r) -> h t b r", b=BL, r=R)
                    for r in range(R):
                        ps = p1_ps.tile([H, TBLK * BL], F32, tag="p1psum")
                        nc.tensor.matmul(
                            ps[:],
                            Wih_s[:, r * H:(r + 1) * H],
                            xT_s[:, tb * TBLK * BL:(tb + 1) * TBLK * BL],
                            start=True, stop=True,
                        )
                        nc.scalar.activation(
                            out=stage4[:, :, :, r],
                            in_=ps.rearrange("h (t b) -> h t b", b=BL),
                            func=Act.Identity,
                            bias=biasT_s[:, r:r + 1],
                            scale=WSCALE,
                        )
                    for b in range(BL):
                        nc.sync.dma_start(
                            out=inp4d[tb * TBLK:(tb + 1) * TBLK, :, b, 0:R].rearrange(
                                "t h r -> h t r"),
                            in_=stage4[:, :, b, :],
                        )

            # ---------------- Phase 2: recurrence ----------------
            with ExitStack() as p2ctx:
                st_pool = p2ctx.enter_context(tc.tile_pool(name=f"st{_rep}", bufs=3))
                hrm_pool = p2ctx.enter_context(tc.tile_pool(name=f"hrm{_rep}", bufs=2))
                msg_pool = p2ctx.enter_context(tc.tile_pool(name=f"msgp{_rep}", bufs=2))
                inp_pool = p2ctx.enter_context(tc.tile_pool(name=f"inpp{_rep}", bufs=3))
                ps_act = p2ctx.enter_context(
                    tc.tile_pool(name=f"ps_act{_rep}", bufs=2, space="PSUM"))
                ps_msg = p2ctx.enter_context(
                    tc.tile_pool(name=f"ps_msg{_rep}", bufs=2, space="PSUM"))
                ps_pret = p2ctx.enter_context(
                    tc.tile_pool(name=f"ps_pret{_rep}", bufs=2, space="PSUM"))

                Hprev = st_pool.tile([H, BRP], BF, tag="hstate")
                nc.vector.memset(Hprev[:], 0.0)

                for t in range(T):
                    inp_t = inp_pool.tile([H, BRP], BF, tag="inp_t")
                    nc.sync.dma_start(inp_t[:], inp_dram[t, :, :])

                    pa = ps_act.tile([H, BRP], F32, tag="pa")
                    paR = pa.rearrange("h (b r) -> h b r", r=RPAD)
                    HprevR = Hprev.rearrange("h (b r) -> h b r", r=RPAD)

                    # local recurrence, first chunk (only needs the first
                    # tanh half of the previous step).
                    # PSUM start=True clears the whole bank's has_written flags,
                    # so only the first matmul into pa may set it.
                    for r in range(0, 64):
                        nc.tensor.matmul(
                            paR[:, :, r],
                            Whh_s[:, r * H:(r + 1) * H],
                            HprevR[:, :, r],
                            start=(r == 0), stop=False,
                        )

                    # region-major state for msg: PE transpose-mode of the
                    # previous bf16 state, evacuated to SBUF by the DVE
                    Hrm = hrm_pool.tile([RPAD, BL * H], BF, tag="hrm")
                    if t == 0:
                        nc.vector.memset(Hrm[:], 0.0)
                    else:
                        pret = ps_pret.tile([RPAD, BL * H], BF, tag="pret")
                        for b in range(BL):
                            nc.tensor.transpose(
                                pret[:, b * H:(b + 1) * H],
                                Hprev[:, b * RPAD:(b + 1) * RPAD],
                                identB_s[:],
                            )
                        for half in range(2):
                            nc.vector.tensor_copy(
                                out=Hrm[:, half * 2 * H:(half + 1) * 2 * H],
                                in_=pret[:, half * 2 * H:(half + 1) * 2 * H])

                    # message matmuls: pm[h,(b,j)] = Hrm_b^T @ C per b
                    pm = ps_msg.tile([H, BRP], F32, tag="pm")
                    for b in range(BL):
                        nc.tensor.matmul(
                            pm[:, b * RPAD:b * RPAD + R],
                            Hrm[0:R, b * H:(b + 1) * H],
                            C_s[:],
                            start=(b == 0), stop=(b == BL - 1),
                        )

                    for r in range(64, R):
                        nc.tensor.matmul(
                            paR[:, :, r],
                            Whh_s[:, r * H:(r + 1) * H],
                            HprevR[:, :, r],
                            start=False, stop=False,
                        )

                    # PSUM->SBUF copy of msg on DVE in r-halves so the cross
                    # matmuls can start as soon as the first half lands
                    Msg = msg_pool.tile([H, BRP], BF, tag="msg")
                    MsgR = Msg.rearrange("h (b r) -> h b r", r=RPAD)
                    pmR = pm.rearrange("h (b r) -> h b r", r=RPAD)
                    for lo, hi in ((0, 64), (64, RPAD)):
                        nc.vector.tensor_copy(out=MsgR[:, :, lo:hi],
                                              in_=pmR[:, :, lo:hi])
                        for r in range(lo, min(hi, R)):
                            nc.tensor.matmul(
                                paR[:, :, r],
                                Wrhh_s[:, r * H:(r + 1) * H],
                                MsgR[:, :, r],
                                start=False, stop=False,
                            )

                    # input drive: accumulated straight into PSUM by the PE
                    # (pa += I @ inp), so no separate DVE add pass is needed
                    nc.tensor.matmul(pa[:], identB_s[:], inp_t[:],
                                     start=False, stop=True)

                    # state tanh in r-halves straight from PSUM: unblocks the
                    # next step's loc chunk early
                    Hnext = st_pool.tile([H, BRP], BF, tag="hstate")
                    HnextR = Hnext.rearrange("h (b r) -> h b r", r=RPAD)
                    for lo, hi in ((0, 64), (64, RPAD)):
                        nc.scalar.activation(out=HnextR[:, :, lo:hi],
                                             in_=paR[:, :, lo:hi], func=Act.Tanh,
                                             scale=1.0 / WSCALE)
                    nc.scalar.dma_start(out=hist_dram[t, :, :], in_=Hnext[:])
                    Hprev = Hnext

            # ---------------- Phase 3: output projection ----------------
            with ExitStack() as p3ctx:
                p3_hh = p3ctx.enter_context(tc.tile_pool(name=f"p3_hh{_rep}", bufs=2))
                p3_ps = p3ctx.enter_context(
                    tc.tile_pool(name=f"p3_ps{_rep}", bufs=2, space="PSUM"))
                p3_ot = p3ctx.enter_context(tc.tile_pool(name=f"p3_ot{_rep}", bufs=2))
                for g in range(NBLK):
                    hh = p3_hh.tile([H, TBLK * BR], BF, tag="hh")
                    hh4 = hh.rearrange("h (t b r) -> h t b r", b=BL, r=R)
                    for b in range(BL):
                        nc.sync.dma_start(
                            out=hh4[:, :, b, :],
                            in_=hist4d[g * TBLK:(g + 1) * TBLK, :, b, 0:R].rearrange(
                                "t h r -> h t r"),
                        )
                    po = p3_ps.tile([TBLK * BL, O], F32, tag="po")
                    for r in range(R):
                        nc.tensor.matmul(
                            po[:],
                            hh4[:, :, :, r],
                            Wout_s[:, r * O:(r + 1) * O],
                            start=(r == 0), stop=False,
                        )
                    nc.tensor.matmul(po[:], ones_s[:, 0:TBLK * BL], bout_s[:],
                                     start=False, stop=True)
                    ot = p3_ot.tile([TBLK * BL, O], F32, tag="ot")
                    nc.scalar.activation(out=ot[:], in_=po[:], func=Act.Copy, scale=1.0)
                    nc.sync.dma_start(
                        out=out_d[g * TBLK:(g + 1) * TBLK, :, :].rearrange(
                            "t b o -> (t b) o"),
                        in_=ot[:],
                    )

        for _rep in range(NREP):
            with ExitStack() as bctx:
                _emit_body(_rep, bctx)

    nc.compile()
    return nc


def _prep_inputs(x, C, W_ih, W_hh, W_rhh, bias, W_out, b_out):
    bf = ml_dtypes.bfloat16
    f8 = ml_dtypes.float8_e4m3
    shared = {
        "C": np.ascontiguousarray(C).astype(bf),
        "Whh": np.ascontiguousarray(
            W_hh.transpose(1, 0, 2).reshape(H, R * H) * WSCALE).astype(bf),
        "Wrhh": np.ascontiguousarray(
            W_rhh.transpose(1, 0, 2).reshape(H, R * H) * WSCALE).astype(f8),
        "Wih": np.ascontiguousarray(W_ih.transpose(1, 0, 2).reshape(I, R * H)).astype(bf),
        "Wout": np.ascontiguousarray(
            W_out.reshape(R, H, O).transpose(1, 0, 2).reshape(H, R * O)
        ).astype(bf),
        "biasT": np.ascontiguousarray(bias.T * WSCALE).astype(np.float32),
        "bout": np.ascontiguousarray(b_out.reshape(1, O)).astype(bf),
    }
    in_maps = []
    for c in range(NCORES):
        xc = x[:, c * BL:(c + 1) * BL, :]                     # [T, BL, I]
        xT = np.ascontiguousarray(xc.transpose(2, 0, 1).reshape(I, TB)).astype(bf)
        m = dict(shared)
        m["xT"] = xT
        in_maps.append(m)
    return in_maps


def kernel(x, C, W_ih, W_hh, W_rhh, bias, W_out, b_out, _trace=False):
    x = np.asarray(x, np.float32)
    in_maps = _prep_inputs(
        x, np.asarray(C, np.float32), np.asarray(W_ih, np.float32),
        np.asarray(W_hh, np.float32), np.asarray(W_rhh, np.float32),
        np.asarray(bias, np.float32), np.asarray(W_out, np.float32),
        np.asarray(b_out, np.float32),
    )
    if "nc" not in _CACHE:
        _CACHE["nc"] = _build_program()
    nc = _CACHE["nc"]
    res = run_bass_kernel_spmd(nc, in_maps, list(range(NCORES)), trace=_trace)
    out = np.empty((T, B, O), np.float32)
    for c in range(NCORES):
        out[:, c * BL:(c + 1) * BL, :] = res.results[c]["out"]
    if _trace:
        return out, res
    return out

